# revision 14
# baseline (speedup 1.0000x reference)
"""Trainium2 Bass kernel for multi-head causal self-attention.

Problem: X [4, 2048, 1024] fp32, Wq/Wk/Wv/Wo [1024, 1024], H=16 heads, HD=64.
reference: out = softmax_causal((X@Wq) (X@Wk)^T / 8) (X@Wv) merged @ Wo.

Sharding over 8 NeuronCores: core c handles batch b = c // 2 and head group
hg = c % 2 (8 heads each). Each core computes a partial [2048, 1024] output
(its heads' contribution through Wo's row shard); the host sums the two
partials per batch (the tensor-parallel all-reduce, done during unsharding).

v2 design notes (vs the phase-separated baseline):
  * Projections are interleaved with attention at matmul granularity so the
    PE never idles long enough for the HAM clock gate to re-throttle, and
    the ACT engine's exp throughput (the real constraint of the attention
    inner loop) is overlapped with projection matmuls.
  * Scores for both heads of a pair go into one [128, 2, 512] fp32 PSUM
    tile (2 banks) so a single ACTIVATE handles exp for both heads
    (halves ACT instruction overhead).
  * Causal masking: one batched DVE add of a [128, 2, 128] -30000 triangle
    per diagonal k-block; fully-masked leading columns are simply never
    computed (scores, exp, and AV all operate on [rs:512]).
  * Normalization uses reciprocal_approx_fast (~5x faster than the
    microcoded reciprocal) + gpsimd partition_broadcast.
  * PSUM evacuation (AV accumulators -> SBUF) on DVE, not ACT.
  * dc-major first projection so the PE starts as soon as the first X^T
    transpose chunk lands; X^T DMA issues split across the two HWDGE
    queues (sync + act); exp table preloaded via a dummy activation.
"""

import sys

for _p in ("/opt/trn_rl_repo", "/root/.axon_site/_ro/trn_rl_repo"):
    if _p not in sys.path:
        sys.path.insert(0, _p)

import ml_dtypes
import numpy as np

import concourse.bass as bass
import concourse.mybir as mybir
import concourse.tile as tile
from concourse import bacc
from concourse.bass_utils import run_bass_kernel_spmd

F32 = mybir.dt.float32
BF16 = mybir.dt.bfloat16
EXPF = mybir.ActivationFunctionType.Exp

B, S, D, H = 4, 2048, 1024, 16
HD = D // H           # 64
HL = H // 2           # 8 heads per core
DL = HL * HD          # 512 local proj width
NEG = -30000.0        # causal mask additive value (exp underflows to 0)
VW = 65               # AV lhsT width: 64 V cols + ones col (denominator row)


def build_program(s=S, d=D, hl=HL):
    dl = hl * HD
    n_st = s // 128          # s-tiles (128 rows)
    n_dc = d // 128          # d-chunks (projection contraction)
    n_pc = dl // 128         # partition chunks (= head pairs)
    n_q = s // 512           # q-chunks
    n_cc = d // 512          # out column chunks

    nc = bacc.Bacc("TRN2", target_bir_lowering=False, debug=False)

    # X is fed pre-transposed and the weights pre-tiled by the host so every
    # input DMA is plain and contiguous (the XBAR transpose + scatter
    # rearrange DMAs dominated the ramp otherwise).
    XT = nc.dram_tensor("XT", [d, s], BF16, kind="ExternalInput")
    WQ = nc.dram_tensor("WQ", [128, n_dc, dl], BF16, kind="ExternalInput")
    WK = nc.dram_tensor("WK", [128, n_dc, dl], BF16, kind="ExternalInput")
    WV = nc.dram_tensor("WV", [128, n_dc, dl], BF16, kind="ExternalInput")
    WO = nc.dram_tensor("WO", [128, n_pc, d], BF16, kind="ExternalInput")
    OUT = nc.dram_tensor("OUT", [s, d], F32, kind="ExternalOutput")

    with tile.TileContext(nc) as tc:
        with tc.tile_pool(name="persist", bufs=1) as persist:
            # [128, 2, 128] additive causal mask for two stacked diagonal
            # blocks: 0 where q >= k else -30000.
            cmask = persist.tile([128, 2, 128], F32)
            nc.gpsimd.memset(cmask[:], 0.0)
            nc.gpsimd.affine_select(
                out=cmask[:], in_=cmask[:],
                compare_op=mybir.AluOpType.is_ge, fill=NEG,
                base=0, pattern=[[0, 2], [1, 128]], channel_multiplier=-1,
            )

            xt = [persist.tile([128, s], BF16, name=f"xt{i}") for i in range(n_dc)]
            wq = persist.tile([128, n_dc, dl], BF16, name="wq")
            wk = persist.tile([128, n_dc, dl], BF16, name="wk")
            wv = persist.tile([128, n_dc, dl], BF16, name="wv")
            wo = persist.tile([128, n_pc, d], BF16, name="wo")
            qt = [persist.tile([128, s], BF16, name=f"qt{i}") for i in range(n_pc)]
            kt = [persist.tile([128, s], BF16, name=f"kt{i}") for i in range(n_pc)]
            vt = [persist.tile([128, hl, VW], BF16, name=f"vt{i}")
                  for i in range(n_st)]
            ot = [persist.tile([128, s], BF16, name=f"ot{i}") for i in range(n_pc)]

            # All input loads ride the scalar HWDGE queue in dependency-
            # priority order (wq + xt0 gate the first matmul); runtime DMAs
            # (dd/sc/OUT) use the sync queue so they never queue behind these.
            nc.scalar.dma_start(wq[:], WQ.ap())
            nc.scalar.dma_start(xt[0][:], XT[0:128, :])
            nc.scalar.dma_start(wk[:], WK.ap())
            for dc in range(1, n_dc):
                nc.scalar.dma_start(
                    xt[dc][:], XT[dc * 128:(dc + 1) * 128, :])
                if dc == 3:
                    nc.scalar.dma_start(wv[:], WV.ap())
            nc.scalar.dma_start(wo[:], WO.ap())

            # exp table preload: a tiny activation during the DMA ramp makes
            # walrus put the ACT_TABLE_LOAD off the critical path of the
            # first real exp.
            scr = persist.tile([128, 8], F32)
            nc.vector.memset(scr[:], 0.0)
            scr2 = persist.tile([128, 8], F32)
            nc.scalar.activation(scr2[:], scr[:], EXPF, scale=1.0)

            # ---- prologue: Q/K projection for j=0 in dc-major order so the
            # PE starts on xt[0] without waiting for the whole transpose.
            with tc.tile_pool(name="prol", bufs=1, space="PSUM") as prol:
                qps = [prol.tile([128, 512], F32, name=f"qps{pc}")
                       for pc in range(n_pc)]
                kps = [prol.tile([128, 512], F32, name=f"kps{pc}")
                       for pc in range(n_pc)]
                for dc in range(n_dc):
                    for pc in range(n_pc):
                        nc.tensor.matmul(
                            qps[pc][:], wq[:, dc, pc * 128:(pc + 1) * 128],
                            xt[dc][:, 0:512],
                            start=(dc == 0), stop=(dc == n_dc - 1))
                        nc.tensor.matmul(
                            kps[pc][:], wk[:, dc, pc * 128:(pc + 1) * 128],
                            xt[dc][:, 0:512],
                            start=(dc == 0), stop=(dc == n_dc - 1))
                for pc in range(n_pc):
                    nc.vector.tensor_copy(qt[pc][:, 0:512], qps[pc][:])
                    nc.vector.tensor_copy(kt[pc][:, 0:512], kps[pc][:])

            with (
                tc.tile_pool(name="pp", bufs=2, space="PSUM") as pp,
                tc.tile_pool(name="sp", bufs=2, space="PSUM") as sp,
                tc.tile_pool(name="avp", bufs=2, space="PSUM") as avp,
                tc.tile_pool(name="work", bufs=3) as work,
                tc.tile_pool(name="norm", bufs=4) as normp,
            ):
                def proj_v(st):
                    ps = pp.tile([128, dl], F32, tag="pp")
                    for dc in range(n_dc):
                        nc.tensor.matmul(
                            ps[:], xt[dc][:, st * 128:(st + 1) * 128],
                            wv[:, dc, :],
                            start=(dc == 0), stop=(dc == n_dc - 1))
                    nc.vector.memset(vt[st][:, :, 64:65], 1.0)
                    nc.vector.tensor_copy(
                        vt[st][:, :, 0:64],
                        ps[:].rearrange("p (h e) -> p h e", h=hl))

                def proj_qk(w, dst, pc, j1):
                    js1 = slice(j1 * 512, (j1 + 1) * 512)
                    ps = pp.tile([128, 512], F32, tag="pp")
                    for dc in range(n_dc):
                        nc.tensor.matmul(
                            ps[:], w[:, dc, pc * 128:(pc + 1) * 128],
                            xt[dc][:, js1],
                            start=(dc == 0), stop=(dc == n_dc - 1))
                    nc.vector.tensor_copy(dst[pc][:, js1], ps[:])

                def out_proj(j, st, cc, pcs, add_to=None, staged=False):
                    """Partial output projection over head pairs `pcs`.
                    Returns the staged SBUF tile (caller DMAs or adds)."""
                    ps = pp.tile([128, 512], F32, tag="pp")
                    for n, pc in enumerate(pcs):
                        nc.tensor.matmul(
                            ps[:], ot[pc][:, st * 128:(st + 1) * 128],
                            wo[:, pc, cc * 512:(cc + 1) * 512],
                            start=(n == 0), stop=(n == len(pcs) - 1))
                    if add_to is None:
                        # the 8 last-chunk partials are all alive at once, so
                        # they get a dedicated 8-deep rotation (a 3-deep one
                        # FIFO-deadlocks DVE behind the final adds).
                        if staged:
                            osb = work.tile([128, 512], F32, tag="osbp",
                                            bufs=8, name=f"osbp{st}_{cc}")
                        else:
                            osb = work.tile([128, 512], F32, tag="osb",
                                            bufs=3, name=f"osb{st}_{cc}")
                        nc.vector.tensor_copy(osb[:], ps[:])
                        return osb
                    nc.vector.tensor_add(add_to[:], add_to[:], ps[:])
                    return add_to

                def dma_out(st, cc, osb):
                    # alternate queues so the final drain isn't serialized
                    # on one HWDGE queue.
                    eng = nc.sync if (st + cc) % 2 == 0 else nc.scalar
                    eng.dma_start(
                        OUT[st * 128:(st + 1) * 128, cc * 512:(cc + 1) * 512],
                        osb[:])

                proj_v_queue = list(range(4))  # vt tiles for j=0 built first
                for st in proj_v_queue:
                    proj_v(st)

                for j in range(n_q):
                    js = slice(j * 512, (j + 1) * 512)
                    osb_partial = {}  # (st, cc) -> staged partial for j == last
                    for pc in range(n_pc):
                        # filler units: always-ready projection work used to
                        # keep the PE busy while ACT exp gates the attention
                        # dependency chain.
                        filler = []
                        if j + 1 < n_q:
                            filler.append(
                                lambda pc=pc, j1=j + 1: proj_qk(wq, qt, pc, j1))
                            filler.append(
                                lambda pc=pc, j1=j + 1: proj_qk(wk, kt, pc, j1))
                            if pc == n_pc - 1:
                                for st in range(4 * (j + 1), 4 * (j + 2)):
                                    filler.append(lambda st=st: proj_v(st))
                        elif pc == n_pc - 1:
                            # last unit of the kernel: stage the partial
                            # output projection over pairs 0..n-2 while pair
                            # n-1 finishes its attention.
                            for st in range(4 * j, 4 * j + 4):
                                for cc in range(n_cc):
                                    def frag(st=st, cc=cc):
                                        osb_partial[(st, cc)] = out_proj(
                                            j, st, cc, list(range(n_pc - 1)),
                                            staged=True)
                                    filler.append(frag)

                        n_i = 4 * j + 4
                        every = max(1, n_i // max(1, len(filler)))
                        av = [avp.tile([VW, 512], F32, tag="av",
                                       name=f"av{j}_{pc}_{h}") for h in (0, 1)]
                        ets = {}

                        def emit_av(i):
                            r = i - 4 * j
                            rs = max(r, 0) * 128
                            et = ets.pop(i)
                            for h in (0, 1):
                                nc.tensor.matmul(
                                    av[h][:, rs:512], vt[i][:, 2 * pc + h, :],
                                    et[:, h, rs:512],
                                    start=(i == 0), stop=(i == n_i - 1))

                        for i in range(n_i):
                            r = i - 4 * j
                            rs = max(r, 0) * 128
                            stp = sp.tile([128, 2, 512], F32, tag="sp")
                            for h in (0, 1):
                                nc.tensor.matmul(
                                    stp[:, h, rs:512],
                                    kt[pc][64 * h:64 * h + 64,
                                           i * 128:(i + 1) * 128],
                                    qt[pc][64 * h:64 * h + 64,
                                           j * 512 + rs:(j + 1) * 512],
                                    start=True, stop=True,
                                    tile_position=(64 * h, 0))
                            if r >= 0:
                                nc.vector.tensor_add(
                                    stp[:, :, rs:rs + 128],
                                    stp[:, :, rs:rs + 128], cmask[:])
                            et = work.tile([128, 2, 512], BF16, tag="et",
                                           bufs=4)
                            nc.scalar.activation(
                                et[:, :, rs:512], stp[:, :, rs:512], EXPF,
                                scale=0.125)
                            ets[i] = et
                            if i >= 2:
                                emit_av(i - 2)
                            if filler and i % every == every - 1:
                                filler.pop(0)()
                        emit_av(n_i - 2)
                        emit_av(n_i - 1)
                        for fr in filler:
                            fr()

                        # normalization: denominators live in av row 64.
                        # h=1 first so its SBUF->SBUF partition-shift DMA
                        # overlaps h=0's DVE work; the denominator row is
                        # evacuated first so the shift DMA starts early.
                        for h in (1, 0):
                            orw = normp.tile([VW, 512], F32, tag="orw",
                                             bufs=4, name=f"orw{j}_{pc}_{h}")
                            nc.vector.tensor_copy(orw[64:65, :], av[h][64:65, :])
                            dd = normp.tile([1, 512], F32, tag="dd", bufs=4,
                                            name=f"dd{j}_{pc}_{h}")
                            nc.sync.dma_start(dd[:], orw[64:65, :])
                            nc.vector.tensor_copy(orw[0:64, :], av[h][0:64, :])
                            rr = normp.tile([1, 512], F32, tag="rr", bufs=4,
                                            name=f"rr{j}_{pc}_{h}")
                            nc.vector.reciprocal_approx_fast(rr[:], dd[:])
                            bc = normp.tile([64, 512], F32, tag="bc", bufs=4,
                                            name=f"bc{j}_{pc}_{h}")
                            nc.gpsimd.partition_broadcast(bc[:], rr[:])
                            if h == 0:
                                nc.vector.tensor_mul(
                                    ot[pc][0:64, js], orw[0:64, :], bc[:])
                            else:
                                sc = normp.tile([64, 512], BF16, tag="sc",
                                                bufs=4, name=f"sc{j}_{pc}")
                                nc.vector.tensor_mul(sc[:], orw[0:64, :], bc[:])
                                nc.sync.dma_start(ot[pc][64:128, js], sc[:])

                    # output projection for this q-chunk
                    if j < n_q - 1:
                        for st in range(4 * j, 4 * j + 4):
                            for cc in range(n_cc):
                                osb = out_proj(j, st, cc, list(range(n_pc)))
                                dma_out(st, cc, osb)
                    else:
                        for st in range(4 * j, 4 * j + 4):
                            for cc in range(n_cc):
                                osb = out_proj(j, st, cc, [n_pc - 1],
                                               add_to=osb_partial[(st, cc)])
                                dma_out(st, cc, osb)

    nc.compile()
    return nc


_NC_CACHE = {}


def _get_program():
    key = (S, D, HL)
    if key not in _NC_CACHE:
        _NC_CACHE[key] = build_program()
    return _NC_CACHE[key]


def _bf16(a):
    return np.ascontiguousarray(a.astype(ml_dtypes.bfloat16))


def _wtile(w):
    # [c*128, m] -> [128, c, m]: contraction chunk i lives at [:, i, :]
    c = w.shape[0] // 128
    return np.ascontiguousarray(
        w.reshape(c, 128, w.shape[1]).transpose(1, 0, 2).astype(
            ml_dtypes.bfloat16))


def make_in_maps(X, Wq, Wk, Wv, Wo):
    in_maps = []
    for c in range(8):
        b, hg = c // 2, c % 2
        cs = slice(hg * DL, hg * DL + DL)
        in_maps.append({
            "XT": _bf16(X[b].T),
            "WQ": _wtile(Wq[:, cs]),
            "WK": _wtile(Wk[:, cs]),
            "WV": _wtile(Wv[:, cs]),
            "WO": _wtile(Wo[cs, :]),
        })
    return in_maps


def gather_out(results):
    out = np.empty((B, S, D), dtype=np.float32)
    for b in range(B):
        out[b] = results[2 * b]["OUT"] + results[2 * b + 1]["OUT"]
    return out


def kernel(X, Wq, Wk, Wv, Wo):
    X = np.asarray(X, dtype=np.float32)
    Wq = np.asarray(Wq, dtype=np.float32)
    Wk = np.asarray(Wk, dtype=np.float32)
    Wv = np.asarray(Wv, dtype=np.float32)
    Wo = np.asarray(Wo, dtype=np.float32)

    nc = _get_program()
    in_maps = make_in_maps(X, Wq, Wk, Wv, Wo)
    res = run_bass_kernel_spmd(nc, in_maps, list(range(8)), trace=False)
    return gather_out(res.results)


if __name__ == "__main__":
    rng = np.random.default_rng(0)
    scale = 1.0 / np.sqrt(D)
    inputs = {
        "X": rng.standard_normal((B, S, D), dtype=np.float32),
        "Wq": rng.standard_normal((D, D), dtype=np.float32) * scale,
        "Wk": rng.standard_normal((D, D), dtype=np.float32) * scale,
        "Wv": rng.standard_normal((D, D), dtype=np.float32) * scale,
        "Wo": rng.standard_normal((D, D), dtype=np.float32) * scale,
    }
    out = kernel(**inputs)
    print("kernel output shape:", out.shape)


# revision 16
# speedup vs baseline: 1.1827x; 1.1827x over previous
"""Trainium2 Bass kernel for multi-head causal self-attention.

Problem: X [4, 2048, 1024] fp32, Wq/Wk/Wv/Wo [1024, 1024], H=16 heads, HD=64.
reference: out = softmax_causal((X@Wq) (X@Wk)^T / 8) (X@Wv) merged @ Wo.

Sharding over 8 NeuronCores: core c handles batch b = c // 2 and head group
hg = c % 2 (8 heads each). Each core computes a partial [2048, 1024] output
(its heads' contribution through Wo's row shard); the host sums the two
partials per batch (the tensor-parallel all-reduce, done during unsharding).

v2 design notes (vs the phase-separated baseline):
  * Projections are interleaved with attention at matmul granularity so the
    PE never idles long enough for the HAM clock gate to re-throttle, and
    the ACT engine's exp throughput (the real constraint of the attention
    inner loop) is overlapped with projection matmuls.
  * Scores for both heads of a pair go into one [128, 2, 512] fp32 PSUM
    tile (2 banks) so a single ACTIVATE handles exp for both heads
    (halves ACT instruction overhead).
  * Causal masking: one batched DVE add of a [128, 2, 128] -30000 triangle
    per diagonal k-block; fully-masked leading columns are simply never
    computed (scores, exp, and AV all operate on [rs:512]).
  * Normalization uses reciprocal_approx_fast (~5x faster than the
    microcoded reciprocal) + gpsimd partition_broadcast.
  * PSUM evacuation (AV accumulators -> SBUF) on DVE, not ACT.
  * dc-major first projection so the PE starts as soon as the first X^T
    transpose chunk lands; X^T DMA issues split across the two HWDGE
    queues (sync + act); exp table preloaded via a dummy activation.
"""

import sys

for _p in ("/opt/trn_rl_repo", "/root/.axon_site/_ro/trn_rl_repo"):
    if _p not in sys.path:
        sys.path.insert(0, _p)

import ml_dtypes
import numpy as np

import concourse.bass as bass
import concourse.mybir as mybir
import concourse.tile as tile
from concourse import bacc
from concourse.bass_utils import run_bass_kernel_spmd

F32 = mybir.dt.float32
BF16 = mybir.dt.bfloat16
EXPF = mybir.ActivationFunctionType.Exp

B, S, D, H = 4, 2048, 1024, 16
HD = D // H           # 64
HL = H // 2           # 8 heads per core
DL = HL * HD          # 512 local proj width
NEG = -30000.0        # causal mask additive value (exp underflows to 0)
VW = 65               # AV lhsT width: 64 V cols + ones col (denominator row)


def build_program(s=S, d=D, hl=HL):
    dl = hl * HD
    n_st = s // 128          # s-tiles (128 rows)
    n_dc = d // 128          # d-chunks (projection contraction)
    n_pc = dl // 128         # partition chunks (= head pairs)
    n_q = s // 512           # q-chunks
    n_cc = d // 512          # out column chunks

    nc = bacc.Bacc("TRN2", target_bir_lowering=False, debug=False)

    # X is fed pre-transposed and the weights pre-tiled by the host so every
    # input DMA is plain and contiguous (the XBAR transpose + scatter
    # rearrange DMAs dominated the ramp otherwise).
    XT = nc.dram_tensor("XT", [d, s], BF16, kind="ExternalInput")
    WQ = nc.dram_tensor("WQ", [128, n_dc, dl], BF16, kind="ExternalInput")
    WK = nc.dram_tensor("WK", [128, n_dc, dl], BF16, kind="ExternalInput")
    WV = nc.dram_tensor("WV", [128, n_dc, dl], BF16, kind="ExternalInput")
    WO = nc.dram_tensor("WO", [128, n_pc, d], BF16, kind="ExternalInput")
    OUT = nc.dram_tensor("OUT", [s, d], F32, kind="ExternalOutput")

    with tile.TileContext(nc) as tc:
        with tc.tile_pool(name="persist", bufs=1) as persist:
            # [128, 2, 128] additive causal mask for two stacked diagonal
            # blocks: 0 where q >= k else -30000.
            cmask = persist.tile([128, 2, 128], F32)
            nc.gpsimd.memset(cmask[:], 0.0)
            nc.gpsimd.affine_select(
                out=cmask[:], in_=cmask[:],
                compare_op=mybir.AluOpType.is_ge, fill=NEG,
                base=0, pattern=[[0, 2], [1, 128]], channel_multiplier=-1,
            )

            xt = [persist.tile([128, s], BF16, name=f"xt{i}") for i in range(n_dc)]
            wq = persist.tile([128, n_dc, dl], BF16, name="wq")
            wk = persist.tile([128, n_dc, dl], BF16, name="wk")
            wv = persist.tile([128, n_dc, dl], BF16, name="wv")
            wo = persist.tile([128, n_pc, d], BF16, name="wo")
            qt = [persist.tile([128, s], BF16, name=f"qt{i}") for i in range(n_pc)]
            kt = [persist.tile([128, s], BF16, name=f"kt{i}") for i in range(n_pc)]
            vt = [persist.tile([128, hl, VW], BF16, name=f"vt{i}")
                  for i in range(n_st)]
            ot = [persist.tile([128, s], BF16, name=f"ot{i}") for i in range(n_pc)]

            # All input loads ride the scalar HWDGE queue in dependency-
            # priority order (wq + xt0 gate the first matmul); runtime DMAs
            # (dd/sc/OUT) use the sync queue so they never queue behind these.
            nc.scalar.dma_start(wq[:], WQ.ap())
            nc.scalar.dma_start(xt[0][:], XT[0:128, :])
            nc.scalar.dma_start(wk[:], WK.ap())
            for dc in range(1, n_dc):
                nc.scalar.dma_start(
                    xt[dc][:], XT[dc * 128:(dc + 1) * 128, :])
                if dc == 3:
                    nc.scalar.dma_start(wv[:], WV.ap())
            nc.scalar.dma_start(wo[:], WO.ap())

            # ---- prologue: Q/K projection for j=0 in dc-major order so the
            # PE starts on xt[0] without waiting for the whole transpose.
            with tc.tile_pool(name="prol", bufs=1, space="PSUM") as prol:
                qps = [prol.tile([128, 512], F32, name=f"qps{pc}")
                       for pc in range(n_pc)]
                kps = [prol.tile([128, 512], F32, name=f"kps{pc}")
                       for pc in range(n_pc)]
                for dc in range(n_dc):
                    for pc in range(n_pc):
                        nc.tensor.matmul(
                            qps[pc][:], wq[:, dc, pc * 128:(pc + 1) * 128],
                            xt[dc][:, 0:512],
                            start=(dc == 0), stop=(dc == n_dc - 1))
                        nc.tensor.matmul(
                            kps[pc][:], wk[:, dc, pc * 128:(pc + 1) * 128],
                            xt[dc][:, 0:512],
                            start=(dc == 0), stop=(dc == n_dc - 1))
                for pc in range(n_pc):
                    nc.vector.tensor_copy(qt[pc][:, 0:512], qps[pc][:])
                    nc.vector.tensor_copy(kt[pc][:, 0:512], kps[pc][:])

            with (
                tc.tile_pool(name="pp", bufs=2, space="PSUM") as pp,
                tc.tile_pool(name="sp", bufs=2, space="PSUM") as sp,
                tc.tile_pool(name="avp", bufs=2, space="PSUM") as avp,
                tc.tile_pool(name="work", bufs=3) as work,
                tc.tile_pool(name="norm", bufs=4) as normp,
            ):
                def proj_v(st):
                    ps = pp.tile([128, dl], F32, tag="pp")
                    for dc in range(n_dc):
                        nc.tensor.matmul(
                            ps[:], xt[dc][:, st * 128:(st + 1) * 128],
                            wv[:, dc, :],
                            start=(dc == 0), stop=(dc == n_dc - 1))
                    nc.vector.memset(vt[st][:, :, 64:65], 1.0)
                    nc.vector.tensor_copy(
                        vt[st][:, :, 0:64],
                        ps[:].rearrange("p (h e) -> p h e", h=hl))

                def proj_qk(w, dst, pc, j1):
                    js1 = slice(j1 * 512, (j1 + 1) * 512)
                    ps = pp.tile([128, 512], F32, tag="pp")
                    for dc in range(n_dc):
                        nc.tensor.matmul(
                            ps[:], w[:, dc, pc * 128:(pc + 1) * 128],
                            xt[dc][:, js1],
                            start=(dc == 0), stop=(dc == n_dc - 1))
                    nc.vector.tensor_copy(dst[pc][:, js1], ps[:])

                def out_proj(j, st, cc, pcs, add_to=None, staged=False):
                    """Partial output projection over head pairs `pcs`.
                    Returns the staged SBUF tile (caller DMAs or adds)."""
                    ps = pp.tile([128, 512], F32, tag="pp")
                    for n, pc in enumerate(pcs):
                        nc.tensor.matmul(
                            ps[:], ot[pc][:, st * 128:(st + 1) * 128],
                            wo[:, pc, cc * 512:(cc + 1) * 512],
                            start=(n == 0), stop=(n == len(pcs) - 1))
                    if add_to is None:
                        # the 8 last-chunk partials are all alive at once, so
                        # they get a dedicated 8-deep rotation (a 3-deep one
                        # FIFO-deadlocks DVE behind the final adds).
                        if staged:
                            osb = work.tile([128, 512], F32, tag="osbp",
                                            bufs=8, name=f"osbp{st}_{cc}")
                        else:
                            osb = work.tile([128, 512], F32, tag="osb",
                                            bufs=3, name=f"osb{st}_{cc}")
                        nc.vector.tensor_copy(osb[:], ps[:])
                        return osb
                    nc.vector.tensor_add(add_to[:], add_to[:], ps[:])
                    return add_to

                def dma_out(st, cc, osb):
                    nc.sync.dma_start(
                        OUT[st * 128:(st + 1) * 128, cc * 512:(cc + 1) * 512],
                        osb[:])

                proj_v_queue = list(range(4))  # vt tiles for j=0 built first
                for st in proj_v_queue:
                    proj_v(st)

                for j in range(n_q):
                    js = slice(j * 512, (j + 1) * 512)
                    osb_partial = {}  # (st, cc) -> staged partial for j == last
                    for pc in range(n_pc):
                        # filler units: always-ready projection work used to
                        # keep the PE busy while ACT exp gates the attention
                        # dependency chain.
                        filler = []
                        if j + 1 < n_q:
                            filler.append(
                                lambda pc=pc, j1=j + 1: proj_qk(wq, qt, pc, j1))
                            filler.append(
                                lambda pc=pc, j1=j + 1: proj_qk(wk, kt, pc, j1))
                            if pc == n_pc - 1:
                                for st in range(4 * (j + 1), 4 * (j + 2)):
                                    filler.append(lambda st=st: proj_v(st))
                        elif pc == n_pc - 1:
                            # last unit of the kernel: stage the partial
                            # output projection over pairs 0..n-2 while pair
                            # n-1 finishes its attention.
                            for st in range(4 * j, 4 * j + 4):
                                for cc in range(n_cc):
                                    def frag(st=st, cc=cc):
                                        osb_partial[(st, cc)] = out_proj(
                                            j, st, cc, list(range(n_pc - 1)),
                                            staged=True)
                                    filler.append(frag)

                        n_i = 4 * j + 4
                        every = max(1, n_i // max(1, len(filler)))
                        av = [avp.tile([VW, 512], F32, tag="av",
                                       name=f"av{j}_{pc}_{h}") for h in (0, 1)]
                        ets = {}

                        def emit_av(i):
                            r = i - 4 * j
                            rs = max(r, 0) * 128
                            et = ets.pop(i)
                            for h in (0, 1):
                                nc.tensor.matmul(
                                    av[h][:, rs:512], vt[i][:, 2 * pc + h, :],
                                    et[:, h, rs:512],
                                    start=(i == 0), stop=(i == n_i - 1))

                        for i in range(n_i):
                            r = i - 4 * j
                            rs = max(r, 0) * 128
                            stp = sp.tile([128, 2, 512], F32, tag="sp")
                            for h in (0, 1):
                                nc.tensor.matmul(
                                    stp[:, h, rs:512],
                                    kt[pc][64 * h:64 * h + 64,
                                           i * 128:(i + 1) * 128],
                                    qt[pc][64 * h:64 * h + 64,
                                           j * 512 + rs:(j + 1) * 512],
                                    start=True, stop=True,
                                    tile_position=(64 * h, 0))
                            if r >= 0:
                                nc.vector.tensor_add(
                                    stp[:, :, rs:rs + 128],
                                    stp[:, :, rs:rs + 128], cmask[:])
                            et = work.tile([128, 2, 512], BF16, tag="et",
                                           bufs=4)
                            nc.scalar.activation(
                                et[:, :, rs:512], stp[:, :, rs:512], EXPF,
                                scale=0.125)
                            ets[i] = et
                            if i >= 2:
                                emit_av(i - 2)
                            if filler and i % every == every - 1:
                                filler.pop(0)()
                        emit_av(n_i - 2)
                        emit_av(n_i - 1)
                        for fr in filler:
                            fr()

                        # normalization: denominators live in av row 64.
                        # h=1 first so its SBUF->SBUF partition-shift DMA
                        # overlaps h=0's DVE work; the denominator row is
                        # evacuated first so the shift DMA starts early.
                        for h in (1, 0):
                            orw = normp.tile([VW, 512], F32, tag="orw",
                                             bufs=4, name=f"orw{j}_{pc}_{h}")
                            nc.vector.tensor_copy(orw[64:65, :], av[h][64:65, :])
                            dd = normp.tile([1, 512], F32, tag="dd", bufs=4,
                                            name=f"dd{j}_{pc}_{h}")
                            nc.sync.dma_start(dd[:], orw[64:65, :])
                            nc.vector.tensor_copy(orw[0:64, :], av[h][0:64, :])
                            rr = normp.tile([1, 512], F32, tag="rr", bufs=4,
                                            name=f"rr{j}_{pc}_{h}")
                            nc.vector.reciprocal_approx_fast(rr[:], dd[:])
                            bc = normp.tile([64, 512], F32, tag="bc", bufs=4,
                                            name=f"bc{j}_{pc}_{h}")
                            nc.gpsimd.partition_broadcast(bc[:], rr[:])
                            if h == 0:
                                nc.vector.tensor_mul(
                                    ot[pc][0:64, js], orw[0:64, :], bc[:])
                            else:
                                sc = normp.tile([64, 512], BF16, tag="sc",
                                                bufs=4, name=f"sc{j}_{pc}")
                                nc.vector.tensor_mul(sc[:], orw[0:64, :], bc[:])
                                nc.sync.dma_start(ot[pc][64:128, js], sc[:])

                    # output projection for this q-chunk
                    if j < n_q - 1:
                        for st in range(4 * j, 4 * j + 4):
                            for cc in range(n_cc):
                                osb = out_proj(j, st, cc, list(range(n_pc)))
                                dma_out(st, cc, osb)
                    else:
                        for st in range(4 * j, 4 * j + 4):
                            for cc in range(n_cc):
                                osb = out_proj(j, st, cc, [n_pc - 1],
                                               add_to=osb_partial[(st, cc)])
                                dma_out(st, cc, osb)

    nc.compile()
    return nc


_NC_CACHE = {}


def _get_program():
    key = (S, D, HL)
    if key not in _NC_CACHE:
        _NC_CACHE[key] = build_program()
    return _NC_CACHE[key]


def _bf16(a):
    return np.ascontiguousarray(a.astype(ml_dtypes.bfloat16))


def _wtile(w):
    # [c*128, m] -> [128, c, m]: contraction chunk i lives at [:, i, :]
    c = w.shape[0] // 128
    return np.ascontiguousarray(
        w.reshape(c, 128, w.shape[1]).transpose(1, 0, 2).astype(
            ml_dtypes.bfloat16))


def make_in_maps(X, Wq, Wk, Wv, Wo):
    in_maps = []
    for c in range(8):
        b, hg = c // 2, c % 2
        cs = slice(hg * DL, hg * DL + DL)
        in_maps.append({
            "XT": _bf16(X[b].T),
            "WQ": _wtile(Wq[:, cs]),
            "WK": _wtile(Wk[:, cs]),
            "WV": _wtile(Wv[:, cs]),
            "WO": _wtile(Wo[cs, :]),
        })
    return in_maps


def gather_out(results):
    out = np.empty((B, S, D), dtype=np.float32)
    for b in range(B):
        out[b] = results[2 * b]["OUT"] + results[2 * b + 1]["OUT"]
    return out


def kernel(X, Wq, Wk, Wv, Wo):
    X = np.asarray(X, dtype=np.float32)
    Wq = np.asarray(Wq, dtype=np.float32)
    Wk = np.asarray(Wk, dtype=np.float32)
    Wv = np.asarray(Wv, dtype=np.float32)
    Wo = np.asarray(Wo, dtype=np.float32)

    nc = _get_program()
    in_maps = make_in_maps(X, Wq, Wk, Wv, Wo)
    res = run_bass_kernel_spmd(nc, in_maps, list(range(8)), trace=False)
    return gather_out(res.results)


if __name__ == "__main__":
    rng = np.random.default_rng(0)
    scale = 1.0 / np.sqrt(D)
    inputs = {
        "X": rng.standard_normal((B, S, D), dtype=np.float32),
        "Wq": rng.standard_normal((D, D), dtype=np.float32) * scale,
        "Wk": rng.standard_normal((D, D), dtype=np.float32) * scale,
        "Wv": rng.standard_normal((D, D), dtype=np.float32) * scale,
        "Wo": rng.standard_normal((D, D), dtype=np.float32) * scale,
    }
    out = kernel(**inputs)
    print("kernel output shape:", out.shape)


# revision 21
# speedup vs baseline: 1.2173x; 1.0292x over previous
"""Trainium2 Bass kernel for multi-head causal self-attention.

Problem: X [4, 2048, 1024] fp32, Wq/Wk/Wv/Wo [1024, 1024], H=16 heads, HD=64.
reference: out = softmax_causal((X@Wq) (X@Wk)^T / 8) (X@Wv) merged @ Wo.

Sharding over 8 NeuronCores: core c handles batch b = c // 2 and head group
hg = c % 2 (8 heads each). Each core computes a partial [2048, 1024] output
(its heads' contribution through Wo's row shard); the host sums the two
partials per batch (the tensor-parallel all-reduce, done during unsharding).

v2 design notes (vs the phase-separated baseline):
  * Projections are interleaved with attention at matmul granularity so the
    PE never idles long enough for the HAM clock gate to re-throttle, and
    the ACT engine's exp throughput (the real constraint of the attention
    inner loop) is overlapped with projection matmuls.
  * Scores for both heads of a pair go into one [128, 2, 512] fp32 PSUM
    tile (2 banks) so a single ACTIVATE handles exp for both heads
    (halves ACT instruction overhead).
  * Causal masking: one batched DVE add of a [128, 2, 128] -30000 triangle
    per diagonal k-block; fully-masked leading columns are simply never
    computed (scores, exp, and AV all operate on [rs:512]).
  * Normalization uses reciprocal_approx_fast (~5x faster than the
    microcoded reciprocal) + gpsimd partition_broadcast.
  * PSUM evacuation (AV accumulators -> SBUF) on DVE, not ACT.
  * dc-major first projection so the PE starts as soon as the first X^T
    transpose chunk lands; X^T DMA issues split across the two HWDGE
    queues (sync + act); exp table preloaded via a dummy activation.
"""

import sys

for _p in ("/opt/trn_rl_repo", "/root/.axon_site/_ro/trn_rl_repo"):
    if _p not in sys.path:
        sys.path.insert(0, _p)

import ml_dtypes
import numpy as np

import concourse.bass as bass
import concourse.mybir as mybir
import concourse.tile as tile
from concourse import bacc
from concourse.bass_utils import run_bass_kernel_spmd

F32 = mybir.dt.float32
BF16 = mybir.dt.bfloat16
EXPF = mybir.ActivationFunctionType.Exp

B, S, D, H = 4, 2048, 1024, 16
HD = D // H           # 64
HL = H // 2           # 8 heads per core
DL = HL * HD          # 512 local proj width
NEG = -30000.0        # causal mask additive value (exp underflows to 0)
VW = 65               # AV lhsT width: 64 V cols + ones col (denominator row)


def build_program(s=S, d=D, hl=HL):
    dl = hl * HD
    n_st = s // 128          # s-tiles (128 rows)
    n_dc = d // 128          # d-chunks (projection contraction)
    n_pc = dl // 128         # partition chunks (= head pairs)
    n_q = s // 512           # q-chunks
    n_cc = d // 512          # out column chunks

    nc = bacc.Bacc("TRN2", target_bir_lowering=False, debug=False)

    # X is fed pre-transposed and the weights pre-tiled by the host so every
    # input DMA is plain and contiguous (the XBAR transpose + scatter
    # rearrange DMAs dominated the ramp otherwise).
    XT = nc.dram_tensor("XT", [d, s], BF16, kind="ExternalInput")
    WQ = nc.dram_tensor("WQ", [128, n_dc, dl], BF16, kind="ExternalInput")
    WK = nc.dram_tensor("WK", [128, n_dc, dl], BF16, kind="ExternalInput")
    WV = nc.dram_tensor("WV", [128, n_dc, dl], BF16, kind="ExternalInput")
    WO = nc.dram_tensor("WO", [128, n_pc, d], BF16, kind="ExternalInput")
    OUT = nc.dram_tensor("OUT", [s, d], F32, kind="ExternalOutput")

    with tile.TileContext(nc) as tc:
        with tc.tile_pool(name="persist", bufs=1) as persist:
            # [128, 2, 128] additive causal mask for two stacked diagonal
            # blocks: 0 where q >= k else -30000.
            cmask = persist.tile([128, 2, 128], F32)
            nc.gpsimd.memset(cmask[:], 0.0)
            nc.gpsimd.affine_select(
                out=cmask[:], in_=cmask[:],
                compare_op=mybir.AluOpType.is_ge, fill=NEG,
                base=0, pattern=[[0, 2], [1, 128]], channel_multiplier=-1,
            )

            xt = [persist.tile([128, s], BF16, name=f"xt{i}") for i in range(n_dc)]
            wq = persist.tile([128, n_dc, dl], BF16, name="wq")
            wk = persist.tile([128, n_dc, dl], BF16, name="wk")
            wv = persist.tile([128, n_dc, dl], BF16, name="wv")
            wo = persist.tile([128, n_pc, d], BF16, name="wo")
            qt = [persist.tile([128, s], BF16, name=f"qt{i}") for i in range(n_pc)]
            kt = [persist.tile([128, s], BF16, name=f"kt{i}") for i in range(n_pc)]
            vt = [persist.tile([128, hl, VW], BF16, name=f"vt{i}")
                  for i in range(n_st)]
            ot = [persist.tile([128, s], BF16, name=f"ot{i}") for i in range(n_pc)]

            # All input loads ride the scalar HWDGE queue in dependency-
            # priority order (wq + xt0 gate the first matmul); runtime DMAs
            # (dd/sc/OUT) use the sync queue so they never queue behind these.
            nc.scalar.dma_start(wq[:], WQ.ap())
            nc.scalar.dma_start(xt[0][:], XT[0:128, :])
            nc.scalar.dma_start(wk[:], WK.ap())
            for dc in range(1, n_dc):
                nc.scalar.dma_start(
                    xt[dc][:], XT[dc * 128:(dc + 1) * 128, :])
                if dc == 3:
                    nc.scalar.dma_start(wv[:], WV.ap())
            nc.scalar.dma_start(wo[:], WO.ap())

            # ---- prologue: Q/K projection for j=0 in dc-major order so the
            # PE starts on xt[0] without waiting for the whole transpose.
            with tc.tile_pool(name="prol", bufs=1, space="PSUM") as prol:
                qps = [prol.tile([128, 512], F32, name=f"qps{pc}")
                       for pc in range(n_pc)]
                kps = [prol.tile([128, 512], F32, name=f"kps{pc}")
                       for pc in range(n_pc)]
                for dc in range(n_dc):
                    for pc in range(n_pc):
                        nc.tensor.matmul(
                            qps[pc][:], wq[:, dc, pc * 128:(pc + 1) * 128],
                            xt[dc][:, 0:512],
                            start=(dc == 0), stop=(dc == n_dc - 1))
                        nc.tensor.matmul(
                            kps[pc][:], wk[:, dc, pc * 128:(pc + 1) * 128],
                            xt[dc][:, 0:512],
                            start=(dc == 0), stop=(dc == n_dc - 1))
                for pc in range(n_pc):
                    nc.vector.tensor_copy(qt[pc][:, 0:512], qps[pc][:])
                    nc.vector.tensor_copy(kt[pc][:, 0:512], kps[pc][:])

            # exp table preload: emitting the first (dummy) activation here
            # makes walrus schedule the ~2.7us ACT_TABLE_LOAD during the
            # PE-heavy prologue instead of on the first attention chain.
            scr = persist.tile([128, 8], F32)
            nc.vector.memset(scr[:], 0.0)
            scr2 = persist.tile([128, 8], F32)
            nc.scalar.activation(scr2[:], scr[:], EXPF, scale=1.0)

            with (
                tc.tile_pool(name="pp", bufs=2, space="PSUM") as pp,
                tc.tile_pool(name="sp", bufs=2, space="PSUM") as sp,
                tc.tile_pool(name="avp", bufs=2, space="PSUM") as avp,
                tc.tile_pool(name="work", bufs=3) as work,
                tc.tile_pool(name="norm", bufs=4) as normp,
            ):
                def proj_v(st):
                    ps = pp.tile([128, dl], F32, tag="pp")
                    for dc in range(n_dc):
                        nc.tensor.matmul(
                            ps[:], xt[dc][:, st * 128:(st + 1) * 128],
                            wv[:, dc, :],
                            start=(dc == 0), stop=(dc == n_dc - 1))
                    nc.vector.memset(vt[st][:, :, 64:65], 1.0)
                    nc.vector.tensor_copy(
                        vt[st][:, :, 0:64],
                        ps[:].rearrange("p (h e) -> p h e", h=hl))

                def proj_qk(w, dst, pc, j1):
                    js1 = slice(j1 * 512, (j1 + 1) * 512)
                    ps = pp.tile([128, 512], F32, tag="pp")
                    for dc in range(n_dc):
                        nc.tensor.matmul(
                            ps[:], w[:, dc, pc * 128:(pc + 1) * 128],
                            xt[dc][:, js1],
                            start=(dc == 0), stop=(dc == n_dc - 1))
                    nc.vector.tensor_copy(dst[pc][:, js1], ps[:])

                def out_proj(j, st, cc, pcs, add_to=None, staged=False):
                    """Partial output projection over head pairs `pcs`.
                    Returns the staged SBUF tile (caller DMAs or adds)."""
                    ps = pp.tile([128, 512], F32, tag="pp")
                    for n, pc in enumerate(pcs):
                        nc.tensor.matmul(
                            ps[:], ot[pc][:, st * 128:(st + 1) * 128],
                            wo[:, pc, cc * 512:(cc + 1) * 512],
                            start=(n == 0), stop=(n == len(pcs) - 1))
                    if add_to is None:
                        # the 8 last-chunk partials are all alive at once, so
                        # they get a dedicated 8-deep rotation (a 3-deep one
                        # FIFO-deadlocks DVE behind the final adds).
                        if staged:
                            osb = work.tile([128, 512], F32, tag="osbp",
                                            bufs=8, name=f"osbp{st}_{cc}")
                        else:
                            osb = work.tile([128, 512], F32, tag="osb",
                                            bufs=3, name=f"osb{st}_{cc}")
                        nc.vector.tensor_copy(osb[:], ps[:])
                        return osb
                    nc.vector.tensor_add(add_to[:], add_to[:], ps[:])
                    return add_to

                def dma_out(st, cc, osb):
                    nc.sync.dma_start(
                        OUT[st * 128:(st + 1) * 128, cc * 512:(cc + 1) * 512],
                        osb[:])

                proj_v_queue = list(range(4))  # vt tiles for j=0 built first
                for st in proj_v_queue:
                    proj_v(st)

                for j in range(n_q):
                    js = slice(j * 512, (j + 1) * 512)
                    last_j = j == n_q - 1
                    osb_partial = {}  # (st, cc) -> staged partial for j == last
                    n_i = 4 * j + 4

                    # phase-level filler: always-ready PE work (projections
                    # for the next q-chunk, output projection of the previous
                    # one) drip-fed between attention steps so the PE never
                    # starves while ACT exp gates the dependency chain.
                    filler = []
                    if j > 0:
                        jp = j - 1
                        for st in range(4 * jp, 4 * jp + 4):
                            for cc in range(n_cc):
                                def og(st=st, cc=cc, jp=jp):
                                    osb = out_proj(
                                        jp, st, cc, list(range(n_pc)))
                                    dma_out(st, cc, osb)
                                filler.append(og)
                    if j + 1 < n_q:
                        for pc in range(n_pc):
                            filler.append(
                                lambda pc=pc, j1=j + 1: proj_qk(wq, qt, pc, j1))
                            filler.append(
                                lambda pc=pc, j1=j + 1: proj_qk(wk, kt, pc, j1))
                        for st in range(4 * (j + 1), 4 * (j + 2)):
                            filler.append(lambda st=st: proj_v(st))
                    n_filler = len(filler) + (8 if last_j else 0)
                    stride = max(1, (n_pc * n_i) // max(1, n_filler))
                    step_ctr = 0

                    for pc in range(n_pc):
                        if last_j and pc == n_pc - 1:
                            # stage the partial output projection over pairs
                            # 0..n-2 while pair n-1 finishes its attention.
                            for st in range(4 * j, 4 * j + 4):
                                for cc in range(n_cc):
                                    def frag(st=st, cc=cc):
                                        osb_partial[(st, cc)] = out_proj(
                                            j, st, cc, list(range(n_pc - 1)),
                                            staged=True)
                                    filler.append(frag)

                        av = [avp.tile([VW, 512], F32, tag="av",
                                       name=f"av{j}_{pc}_{h}") for h in (0, 1)]
                        ets = {}

                        def emit_av(i):
                            r = i - 4 * j
                            rs = max(r, 0) * 128
                            et = ets.pop(i)
                            for h in (0, 1):
                                nc.tensor.matmul(
                                    av[h][:, rs:512], vt[i][:, 2 * pc + h, :],
                                    et[:, h, rs:512],
                                    start=(i == 0), stop=(i == n_i - 1))

                        for i in range(n_i):
                            r = i - 4 * j
                            rs = max(r, 0) * 128
                            stp = sp.tile([128, 2, 512], F32, tag="sp")
                            for h in (0, 1):
                                nc.tensor.matmul(
                                    stp[:, h, rs:512],
                                    kt[pc][64 * h:64 * h + 64,
                                           i * 128:(i + 1) * 128],
                                    qt[pc][64 * h:64 * h + 64,
                                           j * 512 + rs:(j + 1) * 512],
                                    start=True, stop=True,
                                    tile_position=(64 * h, 0))
                            if r >= 0:
                                nc.vector.tensor_add(
                                    stp[:, :, rs:rs + 128],
                                    stp[:, :, rs:rs + 128], cmask[:])
                            et = work.tile([128, 2, 512], BF16, tag="et",
                                           bufs=4)
                            nc.scalar.activation(
                                et[:, :, rs:512], stp[:, :, rs:512], EXPF,
                                scale=0.125)
                            ets[i] = et
                            if i >= 2:
                                emit_av(i - 2)
                            step_ctr += 1
                            if filler and step_ctr % stride == 0:
                                filler.pop(0)()
                        emit_av(n_i - 2)
                        emit_av(n_i - 1)
                        if last_j and pc == n_pc - 1:
                            while filler:
                                filler.pop(0)()

                        # normalization: denominators live in av row 64.
                        # h=1 first so its SBUF->SBUF partition-shift DMA
                        # overlaps h=0's DVE work. Only the very last unit
                        # evacuates the denominator row first (shortens the
                        # kernel tail); elsewhere a single copy releases the
                        # av PSUM bank as fast as possible.
                        tail_unit = last_j and pc == n_pc - 1
                        for h in (1, 0):
                            orw = normp.tile([VW, 512], F32, tag="orw",
                                             bufs=4, name=f"orw{j}_{pc}_{h}")
                            dd = normp.tile([1, 512], F32, tag="dd", bufs=4,
                                            name=f"dd{j}_{pc}_{h}")
                            if tail_unit:
                                nc.vector.tensor_copy(
                                    orw[64:65, :], av[h][64:65, :])
                                nc.sync.dma_start(dd[:], orw[64:65, :])
                                nc.vector.tensor_copy(
                                    orw[0:64, :], av[h][0:64, :])
                            else:
                                nc.vector.tensor_copy(orw[:], av[h][:])
                                nc.sync.dma_start(dd[:], orw[64:65, :])
                            rr = normp.tile([1, 512], F32, tag="rr", bufs=4,
                                            name=f"rr{j}_{pc}_{h}")
                            nc.vector.reciprocal_approx_fast(rr[:], dd[:])
                            bc = normp.tile([64, 512], F32, tag="bc", bufs=4,
                                            name=f"bc{j}_{pc}_{h}")
                            nc.gpsimd.partition_broadcast(bc[:], rr[:])
                            if h == 0:
                                nc.vector.tensor_mul(
                                    ot[pc][0:64, js], orw[0:64, :], bc[:])
                            else:
                                sc = normp.tile([64, 512], BF16, tag="sc",
                                                bufs=4, name=f"sc{j}_{pc}")
                                nc.vector.tensor_mul(sc[:], orw[0:64, :], bc[:])
                                nc.sync.dma_start(ot[pc][64:128, js], sc[:])

                    # drain any leftover filler; the output projection for
                    # this q-chunk rides the NEXT phase's filler (except the
                    # final chunk, completed from the staged partials here).
                    while filler:
                        filler.pop(0)()
                    if last_j:
                        for st in range(4 * j, 4 * j + 4):
                            for cc in range(n_cc):
                                osb = out_proj(j, st, cc, [n_pc - 1],
                                               add_to=osb_partial[(st, cc)])
                                dma_out(st, cc, osb)

    nc.compile()
    return nc


_NC_CACHE = {}


def _get_program():
    key = (S, D, HL)
    if key not in _NC_CACHE:
        _NC_CACHE[key] = build_program()
    return _NC_CACHE[key]


def _bf16(a):
    return np.ascontiguousarray(a.astype(ml_dtypes.bfloat16))


def _wtile(w):
    # [c*128, m] -> [128, c, m]: contraction chunk i lives at [:, i, :]
    c = w.shape[0] // 128
    return np.ascontiguousarray(
        w.reshape(c, 128, w.shape[1]).transpose(1, 0, 2).astype(
            ml_dtypes.bfloat16))


def make_in_maps(X, Wq, Wk, Wv, Wo):
    in_maps = []
    for c in range(8):
        b, hg = c // 2, c % 2
        cs = slice(hg * DL, hg * DL + DL)
        in_maps.append({
            "XT": _bf16(X[b].T),
            "WQ": _wtile(Wq[:, cs]),
            "WK": _wtile(Wk[:, cs]),
            "WV": _wtile(Wv[:, cs]),
            "WO": _wtile(Wo[cs, :]),
        })
    return in_maps


def gather_out(results):
    out = np.empty((B, S, D), dtype=np.float32)
    for b in range(B):
        out[b] = results[2 * b]["OUT"] + results[2 * b + 1]["OUT"]
    return out


def kernel(X, Wq, Wk, Wv, Wo):
    X = np.asarray(X, dtype=np.float32)
    Wq = np.asarray(Wq, dtype=np.float32)
    Wk = np.asarray(Wk, dtype=np.float32)
    Wv = np.asarray(Wv, dtype=np.float32)
    Wo = np.asarray(Wo, dtype=np.float32)

    nc = _get_program()
    in_maps = make_in_maps(X, Wq, Wk, Wv, Wo)
    res = run_bass_kernel_spmd(nc, in_maps, list(range(8)), trace=False)
    return gather_out(res.results)


if __name__ == "__main__":
    rng = np.random.default_rng(0)
    scale = 1.0 / np.sqrt(D)
    inputs = {
        "X": rng.standard_normal((B, S, D), dtype=np.float32),
        "Wq": rng.standard_normal((D, D), dtype=np.float32) * scale,
        "Wk": rng.standard_normal((D, D), dtype=np.float32) * scale,
        "Wv": rng.standard_normal((D, D), dtype=np.float32) * scale,
        "Wo": rng.standard_normal((D, D), dtype=np.float32) * scale,
    }
    out = kernel(**inputs)
    print("kernel output shape:", out.shape)


# revision 24
# speedup vs baseline: 1.2369x; 1.0161x over previous
"""Trainium2 Bass kernel for multi-head causal self-attention.

Problem: X [4, 2048, 1024] fp32, Wq/Wk/Wv/Wo [1024, 1024], H=16 heads, HD=64.
reference: out = softmax_causal((X@Wq) (X@Wk)^T / 8) (X@Wv) merged @ Wo.

Sharding over 8 NeuronCores: core c handles batch b = c // 2 and head group
hg = c % 2 (8 heads each). Each core computes a partial [2048, 1024] output
(its heads' contribution through Wo's row shard); the host sums the two
partials per batch (the tensor-parallel all-reduce, done during unsharding).

v2 design notes (vs the phase-separated baseline):
  * Projections are interleaved with attention at matmul granularity so the
    PE never idles long enough for the HAM clock gate to re-throttle, and
    the ACT engine's exp throughput (the real constraint of the attention
    inner loop) is overlapped with projection matmuls.
  * Scores for both heads of a pair go into one [128, 2, 512] fp32 PSUM
    tile (2 banks) so a single ACTIVATE handles exp for both heads
    (halves ACT instruction overhead).
  * Causal masking: one batched DVE add of a [128, 2, 128] -30000 triangle
    per diagonal k-block; fully-masked leading columns are simply never
    computed (scores, exp, and AV all operate on [rs:512]).
  * Normalization uses reciprocal_approx_fast (~5x faster than the
    microcoded reciprocal) + gpsimd partition_broadcast.
  * PSUM evacuation (AV accumulators -> SBUF) on DVE, not ACT.
  * dc-major first projection so the PE starts as soon as the first X^T
    transpose chunk lands; X^T DMA issues split across the two HWDGE
    queues (sync + act); exp table preloaded via a dummy activation.
"""

import sys

for _p in ("/opt/trn_rl_repo", "/root/.axon_site/_ro/trn_rl_repo"):
    if _p not in sys.path:
        sys.path.insert(0, _p)

import ml_dtypes
import numpy as np

import concourse.bass as bass
import concourse.mybir as mybir
import concourse.tile as tile
from concourse import bacc
from concourse.bass_utils import run_bass_kernel_spmd

F32 = mybir.dt.float32
BF16 = mybir.dt.bfloat16
EXPF = mybir.ActivationFunctionType.Exp

B, S, D, H = 4, 2048, 1024, 16
HD = D // H           # 64
HL = H // 2           # 8 heads per core
DL = HL * HD          # 512 local proj width
NEG = -30000.0        # causal mask additive value (exp underflows to 0)
VW = 65               # AV lhsT width: 64 V cols + ones col (denominator row)


def build_program(s=S, d=D, hl=HL):
    dl = hl * HD
    n_st = s // 128          # s-tiles (128 rows)
    n_dc = d // 128          # d-chunks (projection contraction)
    n_pc = dl // 128         # partition chunks (= head pairs)
    n_q = s // 512           # q-chunks
    n_cc = d // 512          # out column chunks

    nc = bacc.Bacc("TRN2", target_bir_lowering=False, debug=False)

    # X is fed pre-transposed and the weights pre-tiled by the host so every
    # input DMA is plain and contiguous (the XBAR transpose + scatter
    # rearrange DMAs dominated the ramp otherwise).
    XT = nc.dram_tensor("XT", [d, s], BF16, kind="ExternalInput")
    WQ = nc.dram_tensor("WQ", [128, n_dc, dl], BF16, kind="ExternalInput")
    WK = nc.dram_tensor("WK", [128, n_dc, dl], BF16, kind="ExternalInput")
    WV = nc.dram_tensor("WV", [128, n_dc, dl], BF16, kind="ExternalInput")
    WO = nc.dram_tensor("WO", [128, n_pc, d], BF16, kind="ExternalInput")
    OUT = nc.dram_tensor("OUT", [s, d], F32, kind="ExternalOutput")

    with tile.TileContext(nc) as tc:
        with tc.tile_pool(name="persist", bufs=1) as persist:
            # [128, 2, 128] additive causal mask for two stacked diagonal
            # blocks: 0 where q >= k else -30000.
            cmask = persist.tile([128, 2, 128], F32)
            nc.gpsimd.memset(cmask[:], 0.0)
            nc.gpsimd.affine_select(
                out=cmask[:], in_=cmask[:],
                compare_op=mybir.AluOpType.is_ge, fill=NEG,
                base=0, pattern=[[0, 2], [1, 128]], channel_multiplier=-1,
            )

            xt = [persist.tile([128, s], BF16, name=f"xt{i}") for i in range(n_dc)]
            wq = persist.tile([128, n_dc, dl], BF16, name="wq")
            wk = persist.tile([128, n_dc, dl], BF16, name="wk")
            wv = persist.tile([128, n_dc, dl], BF16, name="wv")
            wo = persist.tile([128, n_pc, d], BF16, name="wo")
            qt = [persist.tile([128, s], BF16, name=f"qt{i}") for i in range(n_pc)]
            kt = [persist.tile([128, s], BF16, name=f"kt{i}") for i in range(n_pc)]
            vt = [persist.tile([128, hl, VW], BF16, name=f"vt{i}")
                  for i in range(n_st)]
            ot = [persist.tile([128, s], BF16, name=f"ot{i}") for i in range(n_pc)]

            # All input loads ride the scalar HWDGE queue in dependency-
            # priority order; runtime DMAs (dd/sc/OUT) use the sync queue so
            # they never queue behind these. X^T comes in per-q-chunk column
            # slices: phase 0 only needs columns [0:512] (1 MB), so the first
            # attention unit unblocks ~20us earlier than with whole-tile
            # loads.
            nc.scalar.dma_start(wq[:], WQ.ap())
            nc.scalar.dma_start(xt[0][:, 0:512], XT[0:128, 0:512])
            nc.scalar.dma_start(wk[:], WK.ap())
            for dc in range(1, n_dc):
                nc.scalar.dma_start(
                    xt[dc][:, 0:512], XT[dc * 128:(dc + 1) * 128, 0:512])
            nc.scalar.dma_start(wv[:], WV.ap())
            for q in range(1, n_q):
                qs = slice(q * 512, (q + 1) * 512)
                for dc in range(n_dc):
                    nc.scalar.dma_start(
                        xt[dc][:, qs], XT[dc * 128:(dc + 1) * 128, qs])
                if q == 1:
                    nc.scalar.dma_start(wo[:], WO.ap())

            # exp table preload: emitting the first (dummy) activation here
            # makes walrus schedule the ~2.7us ACT_TABLE_LOAD during the
            # PE-heavy prologue instead of on the first attention chain.
            scr = persist.tile([128, 8], F32)
            nc.vector.memset(scr[:], 0.0)
            scr2 = persist.tile([128, 8], F32)
            nc.scalar.activation(scr2[:], scr[:], EXPF, scale=1.0)

            with (
                tc.tile_pool(name="pp", bufs=2, space="PSUM") as pp,
                tc.tile_pool(name="sp", bufs=2, space="PSUM") as sp,
                tc.tile_pool(name="avp", bufs=2, space="PSUM") as avp,
                tc.tile_pool(name="work", bufs=3) as work,
                tc.tile_pool(name="norm", bufs=4) as normp,
            ):
                def proj_v(st):
                    ps = pp.tile([128, dl], F32, tag="pp")
                    for dc in range(n_dc):
                        nc.tensor.matmul(
                            ps[:], xt[dc][:, st * 128:(st + 1) * 128],
                            wv[:, dc, :],
                            start=(dc == 0), stop=(dc == n_dc - 1))
                    nc.vector.memset(vt[st][:, :, 64:65], 1.0)
                    nc.vector.tensor_copy(
                        vt[st][:, :, 0:64],
                        ps[:].rearrange("p (h e) -> p h e", h=hl))

                def proj_qk(w, dst, pc, j1):
                    js1 = slice(j1 * 512, (j1 + 1) * 512)
                    ps = pp.tile([128, 512], F32, tag="pp")
                    for dc in range(n_dc):
                        nc.tensor.matmul(
                            ps[:], w[:, dc, pc * 128:(pc + 1) * 128],
                            xt[dc][:, js1],
                            start=(dc == 0), stop=(dc == n_dc - 1))
                    nc.vector.tensor_copy(dst[pc][:, js1], ps[:])

                def out_proj(j, st, cc, pcs, add_to=None, staged=False):
                    """Partial output projection over head pairs `pcs`.
                    Returns the staged SBUF tile (caller DMAs or adds)."""
                    ps = pp.tile([128, 512], F32, tag="pp")
                    for n, pc in enumerate(pcs):
                        nc.tensor.matmul(
                            ps[:], ot[pc][:, st * 128:(st + 1) * 128],
                            wo[:, pc, cc * 512:(cc + 1) * 512],
                            start=(n == 0), stop=(n == len(pcs) - 1))
                    if add_to is None:
                        # the 8 last-chunk partials are all alive at once, so
                        # they get a dedicated 8-deep rotation (a 3-deep one
                        # FIFO-deadlocks DVE behind the final adds).
                        if staged:
                            osb = work.tile([128, 512], F32, tag="osbp",
                                            bufs=8, name=f"osbp{st}_{cc}")
                        else:
                            osb = work.tile([128, 512], F32, tag="osb",
                                            bufs=3, name=f"osb{st}_{cc}")
                        nc.vector.tensor_copy(osb[:], ps[:])
                        return osb
                    nc.vector.tensor_add(add_to[:], add_to[:], ps[:])
                    return add_to

                def dma_out(st, cc, osb):
                    nc.sync.dma_start(
                        OUT[st * 128:(st + 1) * 128, cc * 512:(cc + 1) * 512],
                        osb[:])

                # minimal prologue: just what attn(0, pc0) needs — Q/K for
                # pair 0 and the first four V tiles. The remaining j=0
                # projections ride the phase-0 filler.
                proj_qk(wq, qt, 0, 0)
                proj_qk(wk, kt, 0, 0)
                for st in range(4):
                    proj_v(st)

                for j in range(n_q):
                    js = slice(j * 512, (j + 1) * 512)
                    last_j = j == n_q - 1
                    osb_partial = {}  # (st, cc) -> staged partial for j == last
                    n_i = 4 * j + 4

                    # phase-level filler: always-ready PE work (projections
                    # for the next q-chunk, output projection of the previous
                    # one) drip-fed between attention steps so the PE never
                    # starves while ACT exp gates the dependency chain.
                    filler = []
                    if j == 0:
                        # rest of the j=0 projections, in pc order so each
                        # lands just ahead of its attention unit.
                        for pc in range(1, n_pc):
                            filler.append(
                                lambda pc=pc: proj_qk(wq, qt, pc, 0))
                            filler.append(
                                lambda pc=pc: proj_qk(wk, kt, pc, 0))
                    if j > 0:
                        jp = j - 1
                        for st in range(4 * jp, 4 * jp + 4):
                            for cc in range(n_cc):
                                def og(st=st, cc=cc, jp=jp):
                                    osb = out_proj(
                                        jp, st, cc, list(range(n_pc)))
                                    dma_out(st, cc, osb)
                                filler.append(og)
                    if j + 1 < n_q:
                        for pc in range(n_pc):
                            filler.append(
                                lambda pc=pc, j1=j + 1: proj_qk(wq, qt, pc, j1))
                            filler.append(
                                lambda pc=pc, j1=j + 1: proj_qk(wk, kt, pc, j1))
                        for st in range(4 * (j + 1), 4 * (j + 2)):
                            filler.append(lambda st=st: proj_v(st))
                    n_filler = len(filler) + (8 if last_j else 0)
                    stride = max(1, (n_pc * n_i) // max(1, n_filler))
                    step_ctr = 0

                    for pc in range(n_pc):
                        if last_j and pc == n_pc - 1:
                            # stage the partial output projection over pairs
                            # 0..n-2 while pair n-1 finishes its attention.
                            for st in range(4 * j, 4 * j + 4):
                                for cc in range(n_cc):
                                    def frag(st=st, cc=cc):
                                        osb_partial[(st, cc)] = out_proj(
                                            j, st, cc, list(range(n_pc - 1)),
                                            staged=True)
                                    filler.append(frag)

                        av = [avp.tile([VW, 512], F32, tag="av",
                                       name=f"av{j}_{pc}_{h}") for h in (0, 1)]
                        ets = {}

                        def emit_av(i):
                            r = i - 4 * j
                            rs = max(r, 0) * 128
                            et = ets.pop(i)
                            for h in (0, 1):
                                nc.tensor.matmul(
                                    av[h][:, rs:512], vt[i][:, 2 * pc + h, :],
                                    et[:, h, rs:512],
                                    start=(i == 0), stop=(i == n_i - 1))

                        for i in range(n_i):
                            r = i - 4 * j
                            rs = max(r, 0) * 128
                            stp = sp.tile([128, 2, 512], F32, tag="sp")
                            for h in (0, 1):
                                nc.tensor.matmul(
                                    stp[:, h, rs:512],
                                    kt[pc][64 * h:64 * h + 64,
                                           i * 128:(i + 1) * 128],
                                    qt[pc][64 * h:64 * h + 64,
                                           j * 512 + rs:(j + 1) * 512],
                                    start=True, stop=True,
                                    tile_position=(64 * h, 0))
                            if r >= 0:
                                nc.vector.tensor_add(
                                    stp[:, :, rs:rs + 128],
                                    stp[:, :, rs:rs + 128], cmask[:])
                            et = work.tile([128, 2, 512], BF16, tag="et",
                                           bufs=4)
                            nc.scalar.activation(
                                et[:, :, rs:512], stp[:, :, rs:512], EXPF,
                                scale=0.125)
                            ets[i] = et
                            if i >= 2:
                                emit_av(i - 2)
                            step_ctr += 1
                            if filler and step_ctr % stride == 0:
                                filler.pop(0)()
                        emit_av(n_i - 2)
                        emit_av(n_i - 1)
                        if last_j and pc == n_pc - 1:
                            while filler:
                                filler.pop(0)()

                        # normalization: denominators live in av row 64.
                        # h=1 first so its SBUF->SBUF partition-shift DMA
                        # overlaps h=0's DVE work. Only the very last unit
                        # evacuates the denominator row first (shortens the
                        # kernel tail); elsewhere a single copy releases the
                        # av PSUM bank as fast as possible.
                        tail_unit = last_j and pc == n_pc - 1
                        for h in (1, 0):
                            orw = normp.tile([VW, 512], F32, tag="orw",
                                             bufs=4, name=f"orw{j}_{pc}_{h}")
                            dd = normp.tile([1, 512], F32, tag="dd", bufs=4,
                                            name=f"dd{j}_{pc}_{h}")
                            if tail_unit:
                                nc.vector.tensor_copy(
                                    orw[64:65, :], av[h][64:65, :])
                                nc.sync.dma_start(dd[:], orw[64:65, :])
                                nc.vector.tensor_copy(
                                    orw[0:64, :], av[h][0:64, :])
                            else:
                                nc.vector.tensor_copy(orw[:], av[h][:])
                                nc.sync.dma_start(dd[:], orw[64:65, :])
                            rr = normp.tile([1, 512], F32, tag="rr", bufs=4,
                                            name=f"rr{j}_{pc}_{h}")
                            nc.vector.reciprocal_approx_fast(rr[:], dd[:])
                            bc = normp.tile([64, 512], F32, tag="bc", bufs=4,
                                            name=f"bc{j}_{pc}_{h}")
                            nc.gpsimd.partition_broadcast(bc[:], rr[:])
                            if h == 0:
                                nc.vector.tensor_mul(
                                    ot[pc][0:64, js], orw[0:64, :], bc[:])
                            else:
                                sc = normp.tile([64, 512], BF16, tag="sc",
                                                bufs=4, name=f"sc{j}_{pc}")
                                nc.vector.tensor_mul(sc[:], orw[0:64, :], bc[:])
                                nc.sync.dma_start(ot[pc][64:128, js], sc[:])

                    # drain any leftover filler; the output projection for
                    # this q-chunk rides the NEXT phase's filler (except the
                    # final chunk, completed from the staged partials here).
                    while filler:
                        filler.pop(0)()
                    if last_j:
                        for st in range(4 * j, 4 * j + 4):
                            for cc in range(n_cc):
                                osb = out_proj(j, st, cc, [n_pc - 1],
                                               add_to=osb_partial[(st, cc)])
                                dma_out(st, cc, osb)

    nc.compile()
    return nc


_NC_CACHE = {}


def _get_program():
    key = (S, D, HL)
    if key not in _NC_CACHE:
        _NC_CACHE[key] = build_program()
    return _NC_CACHE[key]


def _bf16(a):
    return np.ascontiguousarray(a.astype(ml_dtypes.bfloat16))


def _wtile(w):
    # [c*128, m] -> [128, c, m]: contraction chunk i lives at [:, i, :]
    c = w.shape[0] // 128
    return np.ascontiguousarray(
        w.reshape(c, 128, w.shape[1]).transpose(1, 0, 2).astype(
            ml_dtypes.bfloat16))


def make_in_maps(X, Wq, Wk, Wv, Wo):
    in_maps = []
    for c in range(8):
        b, hg = c // 2, c % 2
        cs = slice(hg * DL, hg * DL + DL)
        in_maps.append({
            "XT": _bf16(X[b].T),
            "WQ": _wtile(Wq[:, cs]),
            "WK": _wtile(Wk[:, cs]),
            "WV": _wtile(Wv[:, cs]),
            "WO": _wtile(Wo[cs, :]),
        })
    return in_maps


def gather_out(results):
    out = np.empty((B, S, D), dtype=np.float32)
    for b in range(B):
        out[b] = results[2 * b]["OUT"] + results[2 * b + 1]["OUT"]
    return out


def kernel(X, Wq, Wk, Wv, Wo):
    X = np.asarray(X, dtype=np.float32)
    Wq = np.asarray(Wq, dtype=np.float32)
    Wk = np.asarray(Wk, dtype=np.float32)
    Wv = np.asarray(Wv, dtype=np.float32)
    Wo = np.asarray(Wo, dtype=np.float32)

    nc = _get_program()
    in_maps = make_in_maps(X, Wq, Wk, Wv, Wo)
    res = run_bass_kernel_spmd(nc, in_maps, list(range(8)), trace=False)
    return gather_out(res.results)


if __name__ == "__main__":
    rng = np.random.default_rng(0)
    scale = 1.0 / np.sqrt(D)
    inputs = {
        "X": rng.standard_normal((B, S, D), dtype=np.float32),
        "Wq": rng.standard_normal((D, D), dtype=np.float32) * scale,
        "Wk": rng.standard_normal((D, D), dtype=np.float32) * scale,
        "Wv": rng.standard_normal((D, D), dtype=np.float32) * scale,
        "Wo": rng.standard_normal((D, D), dtype=np.float32) * scale,
    }
    out = kernel(**inputs)
    print("kernel output shape:", out.shape)
